# revision 1
# baseline (speedup 1.0000x reference)
"""Causal depthwise conv1d (K=4) + SiLU, sharded over 8 NeuronCores.

Full shapes: x [4, 8192, 2048] f32, weight [2048, 4] f32 -> y [4, 8192, 2048] f32.

Strategy: tensor-parallel over the hidden/channel dim (fully channel
independent, no halo exchange). Each core gets 256 channels, reorganized
host-side to channel-major [B*256, 3+S] (3 leading zero columns provide the
causal padding) so the conv runs along the free dim with channels on SBUF
partitions.

Compute: all 4 taps run on the TensorEngine as float32r diagonal-matrix
matmuls accumulating in PSUM (psum[c,t] += w_i[c] * x[c, t-3+i] via
diag(w_i) @ x_shifted). The diagonal weight matrices are built on-device
(gpsimd affine_select identity mask x per-partition tap scalar on DVE). DVE
rounds each input tile fp32 -> fp32r (the fp32r matmul contract requires
rounded producers); ACT applies SiLU straight out of PSUM and triggers the
output DMA on its own HWDGE ring, with inputs streaming on SP's ring.

Raw bass (no Tile framework): the installed walrus codegen only accepts one
sync wait per compute instruction, so all synchronization is explicit wait_ge
sequencer instructions. Per-buffer-slot DMA semaphores keep concurrent DMA
completion increments unambiguous. Sem increments fire at instruction
completion, but the sequencer runs ahead, so consumers of an engine's result
always gate on that completion increment (including same-engine self-waits
before DMA triggers).
"""

import contextlib

import numpy as np

B, S, H, K = 4, 8192, 2048, 4
N_CORES = 8
HC = H // N_CORES          # 256 channels per core
ROWS = B * HC              # 1024 rows per core, row r = b*HC + c
NU = ROWS // 128           # 8 partition units
T = 2048                   # token tile
NT = S // T
NTILES = NU * NT           # 32
NB = 6                     # buffers per tile kind
NC_CHUNK = 512             # one PSUM bank of fp32
NCHUNKS = T // NC_CHUNK

_last_results = None       # test harness introspection (exec_time_ns etc.)
_ACT_FUNC = "Silu"         # sim override hook (CoreSim lacks Silu)


def _build_program():
    from concourse import bass, mybir

    f32 = mybir.dt.float32
    f32r = mybir.dt.float32r
    AF = mybir.ActivationFunctionType

    nc = bass.Bass()
    # x arrives with 3 leading zero columns (causal padding): [ROWS, 3+S]
    x_d = nc.declare_dram_parameter("x", [ROWS, S + 3], f32, isOutput=False)
    w_d = nc.declare_dram_parameter("w", [128, NU * K + 1], f32, isOutput=False)
    y_d = nc.declare_dram_parameter("y", [ROWS, S], f32, isOutput=True)

    with contextlib.ExitStack() as st:
        wt = st.enter_context(nc.sbuf_tensor("wt", [128, NU * K + 1], f32))
        eye = st.enter_context(nc.sbuf_tensor("eye", [128, 128], f32))
        wtr = st.enter_context(nc.sbuf_tensor("wtr", [128, NU * K * 128], f32r))
        xts = [
            st.enter_context(nc.sbuf_tensor(f"xt{i}", [128, T + 3], f32))
            for i in range(NB)
        ]
        xrs = [
            st.enter_context(nc.sbuf_tensor(f"xr{i}", [128, T + 3], f32r))
            for i in range(NB)
        ]
        yts = [
            st.enter_context(nc.sbuf_tensor(f"yt{i}", [128, T], f32))
            for i in range(NB)
        ]
        pss = [
            st.enter_context(nc.psum_tensor(f"ps{i}", [128, T], f32))
            for i in range(2)
        ]
        zb = wt[:, NU * K : NU * K + 1]           # zeros column (Silu bias)

        def wdiag(k, i):
            u = k // NT
            c0 = (u * K + i) * 128
            return wtr[:, c0 : c0 + 128]

        def x_rows(k):
            r0 = (k // NT) * 128
            return r0, r0 + 128

        with (
            nc.Block() as block,
            nc.semaphore("wsem") as wsem,
            nc.semaphore("esem") as esem,
            nc.semaphore("act") as act,
            nc.semaphore("dve") as dve,
            nc.semaphore("pe") as pe,
            contextlib.ExitStack() as sems,
        ):
            din = [
                sems.enter_context(nc.semaphore(f"din{i}")) for i in range(NB)
            ]
            dout = [
                sems.enter_context(nc.semaphore(f"dout{i}")) for i in range(NB)
            ]

            @block.sync
            def _(sync):
                sync.dma_start(out=wt[:, :], in_=w_d[:, :]).then_inc(wsem, 16)
                for k in range(NTILES):
                    r0, r1 = x_rows(k)
                    t0 = (k % NT) * T
                    if k >= NB:
                        # xt slot free once DVE rounded tile k-NB out of it
                        prev = k - NB
                        sync.wait_ge(dve, 2 if prev == 0 else prev + 3)
                    # padded coords: window [t0-3, t0+T) = x_d cols [t0, t0+T+3)
                    sync.dma_start(
                        out=xts[k % NB][:, :],
                        in_=x_d[r0:r1, t0 : t0 + T + 3],
                    ).then_inc(din[k % NB], 16)

            @block.gpsimd
            def _(gpsimd):
                # identity mask for the diagonal weight build (affine_select
                # only exists on gpsimd)
                gpsimd.memset(eye[:, :], 1.0)
                gpsimd.affine_select(
                    out=eye[:, :], in_=eye[:, :],
                    pattern=[[1, 128]], base=0, channel_multiplier=-1,
                    compare_op=mybir.AluOpType.is_equal, fill=0.0,
                ).then_inc(esem)

            @block.vector
            def _(vector):
                # build the diagonal weight matrices on-device; unit 0 first
                # so PE can start, the rest behind tile 0's rounding.
                # dve incs: #1 unit-0 diags, #2 round_0, #3 remaining diags,
                # #k+3 round_k (k>=1)
                def diag_build(units):
                    for u in units:
                        for i in range(K):
                            mul = vector.tensor_scalar_mul(
                                wtr[:, (u * K + i) * 128 : (u * K + i + 1) * 128],
                                eye[:, :],
                                wt[:, u * K + i : u * K + i + 1],
                            )
                    return mul

                def round_tile(k):
                    vector.wait_ge(din[k % NB], 16 * (k // NB + 1))
                    if k >= NB:
                        # xr slot free once PE consumed tile k-NB
                        vector.wait_ge(pe, k - NB + 1)
                    return vector.tensor_copy(
                        out=xrs[k % NB][:, :], in_=xts[k % NB][:, :]
                    )

                vector.wait_ge(wsem, 16)
                vector.wait_ge(esem, 1)
                diag_build([0]).then_inc(dve)
                round_tile(0).then_inc(dve)
                diag_build(range(1, NU)).then_inc(dve)
                for k in range(1, NTILES):
                    round_tile(k).then_inc(dve)

            @block.tensor
            def _(tensor):
                for k in range(NTILES):
                    # k=0: unit-0 diags + round_0; k>=1: all diags + round_k
                    tensor.wait_ge(dve, 2 if k == 0 else k + 3)
                    if k >= 2:
                        # psum buffer free once silu of tile k-2 done
                        tensor.wait_ge(act, k - 1)
                    ps = pss[k % 2]
                    xr = xrs[k % NB]
                    for c in range(NCHUNKS):
                        c0 = c * NC_CHUNK
                        for i in range(K):
                            mm = tensor.matmul(
                                ps[:, c0 : c0 + NC_CHUNK],
                                wdiag(k, i),
                                xr[:, c0 + i : c0 + i + NC_CHUNK],
                                start=(i == 0),
                                stop=(i == K - 1),
                                skip_group_check=True,
                            )
                    mm.then_inc(pe)

            @block.scalar
            def _(scalar):
                func = getattr(AF, _ACT_FUNC)
                for k in range(NTILES):
                    scalar.wait_ge(pe, k + 1)
                    if k >= NB:
                        # yt slot's previous store (tile k-NB) must be done
                        scalar.wait_ge(dout[k % NB], 16 * (k // NB))
                    scalar.activation(
                        out=yts[k % NB][:, :], in_=pss[k % 2][:, :],
                        func=func,
                        bias=0.0 if func == AF.Copy else zb,
                        scale=1.0,
                    ).then_inc(act)
                    # the DMA trigger races ahead of the still-streaming
                    # activation write; self-wait on its completion inc
                    scalar.wait_ge(act, k + 1)
                    r0, r1 = x_rows(k)
                    t0 = (k % NT) * T
                    scalar.dma_start(
                        out=y_d[r0:r1, t0 : t0 + T], in_=yts[k % NB][:, :]
                    ).then_inc(dout[k % NB], 16)
                for i in range(NB):
                    n_stores = len([k for k in range(NTILES) if k % NB == i])
                    scalar.wait_ge(dout[i], 16 * n_stores)

    return nc


def kernel(x, weight):
    global _last_results
    from concourse.bass_utils import run_bass_kernel_spmd

    x = np.asarray(x, dtype=np.float32)
    weight = np.asarray(weight, dtype=np.float32)

    nc = _build_program()

    in_maps = []
    for core in range(N_CORES):
        sl = slice(core * HC, (core + 1) * HC)
        # [B, S, HC] -> [B, HC, S] -> [ROWS, S] with 3 leading zero columns
        # (the causal padding), row r = b*HC + c
        xs = np.zeros((ROWS, S + 3), np.float32)
        xs[:, 3:] = x[:, :, sl].transpose(0, 2, 1).reshape(ROWS, S)
        ws = weight[sl, :]  # (HC, K)
        w_host = np.zeros((128, NU * K + 1), np.float32)
        for u in range(NU):
            blk = u % (HC // 128)
            w_host[:, u * K : (u + 1) * K] = ws[blk * 128 : (blk + 1) * 128, :]
        in_maps.append({"x": xs, "w": w_host})

    res = run_bass_kernel_spmd(nc, in_maps, list(range(N_CORES)))
    _last_results = res

    out = np.empty((B, S, H), np.float32)
    for core in range(N_CORES):
        sl = slice(core * HC, (core + 1) * HC)
        yc = res.results[core]["y"].reshape(B, HC, S)
        out[:, :, sl] = yc.transpose(0, 2, 1)
    return out



# revision 3
# speedup vs baseline: 1.6925x; 1.6925x over previous
"""Causal depthwise conv1d (K=4) + SiLU, sharded over 8 NeuronCores.

Full shapes: x [4, 8192, 2048] f32, weight [2048, 4] f32 -> y [4, 8192, 2048] f32.

Strategy: tensor-parallel over the hidden/channel dim (fully channel
independent, no halo exchange). Each core gets 256 channels -> 1024
independent rows (batch x channel). All HBM traffic is bf16 (the 2e-2
rel-err budget dwarfs bf16's ~1e-3), halving the memory-bound roofline
vs f32.

Layout: time is phase-split host-side, t = 4j + p. SBUF partition dim
packs (32 rows x 4 phases); the free dim is the block index j. A causal
conv tap then only ever reads the current block j or block j-1, so each
512-block PSUM chunk needs just TWO 128x128 banded-matmul accumulations
(prev-block taps + cur-block taps) instead of one diag matmul per tap:
2x less TensorEngine time than the diagonal formulation, keeping PE
(~55us) under the bf16 DMA roofline (~94us). The banded weight matrices
(block-diagonal over rows, 4x4 tap bands over phases) are built host-side
and DMA'd once. A leading zero block column provides causal padding.

Compute: PE accumulates bf16 matmuls into f32 PSUM; ACT applies SiLU
straight out of PSUM, writing bf16, and triggers the output DMA on its
own HWDGE ring, with inputs streaming on SP's ring. DVE/GPSIMD are idle.

Raw bass (no Tile framework): the installed walrus codegen only accepts one
sync wait per compute instruction, so all synchronization is explicit wait_ge
sequencer instructions. Per-buffer-slot DMA semaphores keep concurrent DMA
completion increments unambiguous. Sem increments fire at instruction
completion, but the sequencer runs ahead, so consumers of an engine's result
always gate on that completion increment (including same-engine self-waits
before DMA triggers).
"""

import contextlib

import numpy as np
import ml_dtypes

B, S, H, K = 4, 8192, 2048, 4
N_CORES = 8
HC = H // N_CORES          # 256 channels per core
ROWS = B * HC              # 1024 rows per core, row r = b*HC + c
P = 4                      # time phases per partition group, t = P*j + p
J = S // P                 # 2048 blocks
RPU = 128 // P             # 32 rows per partition unit
NU = ROWS // RPU           # 32 units (tiles); tile k = unit k, all blocks
NG = HC // RPU             # 8 distinct weight groups (weights repeat per b)
NB = 6                     # buffers per tile kind
NC_CHUNK = 512             # one PSUM bank of fp32
NCHUNKS = J // NC_CHUNK    # 4

BF16 = ml_dtypes.bfloat16

_last_results = None       # test harness introspection (exec_time_ns etc.)
_ACT_FUNC = "Silu"         # sim override hook (CoreSim lacks Silu)


def _build_program():
    from concourse import bass, mybir

    f32 = mybir.dt.float32
    bf16 = mybir.dt.bfloat16
    AF = mybir.ActivationFunctionType

    nc = bass.Bass()
    # phase-split x: row 128*u + 4*rho + p holds x[32u+rho, P*j+p] at col 1+j;
    # col 0 is the causal zero block.
    x_d = nc.declare_dram_parameter("x", [NU * 128, J + 1], bf16, isOutput=False)
    # 16 stationary matrices: [Wprev_g | Wcur_g] for g in 0..NG
    w_d = nc.declare_dram_parameter("w", [128, NG * 2 * 128], bf16, isOutput=False)
    z_d = nc.declare_dram_parameter("z", [128, 1], f32, isOutput=False)
    y_d = nc.declare_dram_parameter("y", [NU * 128, J], bf16, isOutput=True)

    with contextlib.ExitStack() as st:
        wsb = st.enter_context(nc.sbuf_tensor("wsb", [128, NG * 2 * 128], bf16))
        zsb = st.enter_context(nc.sbuf_tensor("zsb", [128, 1], f32))
        xts = [
            st.enter_context(nc.sbuf_tensor(f"xt{i}", [128, J + 1], bf16))
            for i in range(NB)
        ]
        yts = [
            st.enter_context(nc.sbuf_tensor(f"yt{i}", [128, J], bf16))
            for i in range(NB)
        ]
        pss = [
            st.enter_context(nc.psum_tensor(f"ps{i}", [128, J], f32))
            for i in range(2)
        ]

        def wmat(g, which):               # which: 0=prev-block, 1=cur-block
            c0 = (g * 2 + which) * 128
            return wsb[:, c0 : c0 + 128]

        with (
            nc.Block() as block,
            nc.semaphore("wsem") as wsem,
            nc.semaphore("act") as act,
            nc.semaphore("pe") as pe,
            contextlib.ExitStack() as sems,
        ):
            din = [
                sems.enter_context(nc.semaphore(f"din{i}")) for i in range(NB)
            ]
            dout = [
                sems.enter_context(nc.semaphore(f"dout{i}")) for i in range(NB)
            ]

            @block.sync
            def _(sync):
                sync.dma_start(out=wsb[:, :], in_=w_d[:, :]).then_inc(wsem, 16)
                sync.dma_start(out=zsb[:, :], in_=z_d[:, :]).then_inc(wsem, 16)
                for k in range(NU):
                    if k >= NB:
                        # xt slot free once PE consumed tile k-NB
                        sync.wait_ge(pe, k - NB + 1)
                    sync.dma_start(
                        out=xts[k % NB][:, :],
                        in_=x_d[k * 128 : (k + 1) * 128, :],
                    ).then_inc(din[k % NB], 16)

            @block.tensor
            def _(tensor):
                tensor.wait_ge(wsem, 32)
                for k in range(NU):
                    tensor.wait_ge(din[k % NB], 16 * (k // NB + 1))
                    if k >= 2:
                        # psum buffer free once silu of tile k-2 done
                        tensor.wait_ge(act, k - 1)
                    ps = pss[k % 2]
                    xt = xts[k % NB]
                    g = k % NG
                    for c in range(NCHUNKS):
                        c0 = c * NC_CHUNK
                        mm = tensor.matmul(
                            ps[:, c0 : c0 + NC_CHUNK],
                            wmat(g, 0),
                            xt[:, c0 : c0 + NC_CHUNK],
                            start=True,
                            stop=False,
                            skip_group_check=True,
                        )
                        mm = tensor.matmul(
                            ps[:, c0 : c0 + NC_CHUNK],
                            wmat(g, 1),
                            xt[:, c0 + 1 : c0 + 1 + NC_CHUNK],
                            start=False,
                            stop=True,
                            skip_group_check=True,
                        )
                    mm.then_inc(pe)

            @block.scalar
            def _(scalar):
                func = getattr(AF, _ACT_FUNC)
                for k in range(NU):
                    scalar.wait_ge(pe, k + 1)
                    if k >= NB:
                        # yt slot's previous store (tile k-NB) must be done
                        scalar.wait_ge(dout[k % NB], 16 * (k // NB))
                    scalar.activation(
                        out=yts[k % NB][:, :], in_=pss[k % 2][:, :],
                        func=func,
                        bias=0.0 if func == AF.Copy else zsb[:, :],
                        scale=1.0,
                    ).then_inc(act)
                    # the DMA trigger races ahead of the still-streaming
                    # activation write; self-wait on its completion inc
                    scalar.wait_ge(act, k + 1)
                    scalar.dma_start(
                        out=y_d[k * 128 : (k + 1) * 128, :],
                        in_=yts[k % NB][:, :],
                    ).then_inc(dout[k % NB], 16)
                for i in range(NB):
                    n_stores = len([k for k in range(NU) if k % NB == i])
                    scalar.wait_ge(dout[i], 16 * n_stores)

    return nc


def _weight_tables(ws):
    """ws: (HC, K) f32 -> [128, NG*2*128] bf16 banded stationary matrices.

    Partition index q = 4*rho + p. For output y[r, P*j+p_out], the cur-block
    matmul reads x[r, P*j+p_in] with tap i = p_in - p_out + 3 (p_in <= p_out);
    the prev-block matmul reads x[r, P*(j-1)+p_in] with tap i = p_in - p_out - 1
    (p_in > p_out). matmul computes out[q_out] = sum_q_in lhsT[q_in, q_out]*rhs.
    """
    wtab = np.zeros((128, NG * 2 * 128), np.float32)
    rho = np.arange(RPU)
    for g in range(NG):
        ch = ws[RPU * g : RPU * (g + 1)]          # (RPU, K)
        Wp = np.zeros((128, 128), np.float32)
        Wc = np.zeros((128, 128), np.float32)
        for p_in in range(P):
            for p_out in range(P):
                i = p_in - p_out + (K - 1)
                if 0 <= i < K:
                    Wc[P * rho + p_in, P * rho + p_out] = ch[:, i]
                i = p_in - p_out - 1
                if 0 <= i < K:
                    Wp[P * rho + p_in, P * rho + p_out] = ch[:, i]
        wtab[:, (2 * g) * 128 : (2 * g + 1) * 128] = Wp
        wtab[:, (2 * g + 1) * 128 : (2 * g + 2) * 128] = Wc
    return wtab.astype(BF16)


def kernel(x, weight):
    global _last_results
    from concourse.bass_utils import run_bass_kernel_spmd

    x = np.asarray(x, dtype=np.float32)
    weight = np.asarray(weight, dtype=np.float32)

    nc = _build_program()

    zeros = np.zeros((128, 1), np.float32)
    in_maps = []
    for core in range(N_CORES):
        sl = slice(core * HC, (core + 1) * HC)
        # [B, S, HC] -> [B, HC, S] -> [ROWS, S], row r = b*HC + c
        xc = x[:, :, sl].transpose(0, 2, 1).reshape(ROWS, S)
        # phase split: row 4r+p, col j = x[r, 4j+p]; leading zero block col
        xs = np.zeros((ROWS * P, J + 1), BF16)
        xs[:, 1:] = xc.reshape(ROWS, J, P).transpose(0, 2, 1).reshape(ROWS * P, J)
        in_maps.append({"x": xs, "w": _weight_tables(weight[sl, :]), "z": zeros})

    res = run_bass_kernel_spmd(nc, in_maps, list(range(N_CORES)))
    _last_results = res

    out = np.empty((B, S, H), np.float32)
    for core in range(N_CORES):
        sl = slice(core * HC, (core + 1) * HC)
        yc = np.asarray(res.results[core]["y"], dtype=np.float32)
        # undo phase split, then row-major [B, HC, S] -> [B, S, HC]
        yc = yc.reshape(ROWS, P, J).transpose(0, 2, 1).reshape(B, HC, S)
        out[:, :, sl] = yc.transpose(0, 2, 1)
    return out


# revision 13
# speedup vs baseline: 1.8685x; 1.1040x over previous
"""Causal depthwise conv1d (K=4) + SiLU, sharded over 8 NeuronCores.

Full shapes: x [4, 8192, 2048] f32, weight [2048, 4] f32 -> y [4, 8192, 2048] f32.

Strategy: tensor-parallel over the hidden/channel dim (fully channel
independent, no halo exchange). Each core gets 256 channels -> 1024
independent rows (batch x channel). All HBM traffic is bf16 (the 2e-2
rel-err budget dwarfs bf16's ~1e-3), halving the memory-bound roofline
vs f32.

Layout: time is phase-split host-side, t = 4j + p. SBUF partition dim
packs (32 rows x 4 phases); the free dim is the block index j. A causal
conv tap then only ever reads the current block j or block j-1, so each
512-block PSUM chunk needs just TWO 128x128 banded-matmul accumulations
(prev-block taps + cur-block taps) instead of one diag matmul per tap:
2x less TensorEngine time than the diagonal formulation, keeping PE
(~55us) under the bf16 DMA roofline (~94us). The banded weight matrices
(block-diagonal over rows, 4x4 tap bands over phases) are built host-side
and DMA'd once. A leading zero block column provides causal padding.

Compute: PE accumulates bf16 matmuls into f32 PSUM; ACT applies SiLU
straight out of PSUM, writing bf16. Output DMAs are triggered by the
otherwise-idle DVE so the ~900ns semaphore-propagation wait between an
activation and its store never blocks the next activation; inputs stream
on SP's ring. x is stored unpadded (4096B-aligned partition lines — the
odd 2-byte halo column measurably degraded input DMA bandwidth); the
causal zero block is handled by skipping the prev-block matmul's first
output column in chunk 0.

Raw bass (no Tile framework): the installed walrus codegen only accepts one
sync wait per compute instruction, so all synchronization is explicit wait_ge
sequencer instructions. Per-buffer-slot DMA semaphores keep concurrent DMA
completion increments unambiguous. Sem increments fire at instruction
completion, but the sequencer runs ahead, so consumers of an engine's result
always gate on that completion increment (including same-engine self-waits
before DMA triggers).
"""

import contextlib

import numpy as np
import ml_dtypes

B, S, H, K = 4, 8192, 2048, 4
N_CORES = 8
HC = H // N_CORES          # 256 channels per core
ROWS = B * HC              # 1024 rows per core, row r = b*HC + c
P = 4                      # time phases per partition group, t = P*j + p
J = S // P                 # 2048 blocks
RPU = 128 // P             # 32 rows per partition unit
NU = ROWS // RPU           # 32 units (tiles); tile k = unit k, all blocks
NG = HC // RPU             # 8 distinct weight groups (weights repeat per b)
NB = 8                     # buffers per tile kind
NC_CHUNK = 512             # one PSUM bank of fp32
NCHUNKS = J // NC_CHUNK    # 4
PC = 1024                  # psum buffer / activation chunk (2 banks)
NH = J // PC               # 2 chunks per tile
NPS = 4                    # psum buffers (all 8 banks); ping depth 4

BF16 = ml_dtypes.bfloat16

_last_results = None       # test harness introspection (exec_time_ns etc.)
_ACT_FUNC = "Silu"         # sim override hook (CoreSim lacks Silu)


def _build_program():
    from concourse import bass, mybir

    f32 = mybir.dt.float32
    bf16 = mybir.dt.bfloat16
    AF = mybir.ActivationFunctionType

    nc = bass.Bass()
    # phase-split x: row 128*u + 4*rho + p holds x[32u+rho, P*j+p] at col j
    x_d = nc.declare_dram_parameter("x", [NU * 128, J], bf16, isOutput=False)
    # 16 stationary matrices: [Wprev_g | Wcur_g] for g in 0..NG
    w_d = nc.declare_dram_parameter("w", [128, NG * 2 * 128], bf16, isOutput=False)
    z_d = nc.declare_dram_parameter("z", [128, 1], f32, isOutput=False)
    y_d = nc.declare_dram_parameter("y", [NU * 128, J], bf16, isOutput=True)

    with contextlib.ExitStack() as st:
        wsb = st.enter_context(nc.sbuf_tensor("wsb", [128, NG * 2 * 128], bf16))
        zsb = st.enter_context(nc.sbuf_tensor("zsb", [128, 1], f32))
        xts = [
            st.enter_context(nc.sbuf_tensor(f"xt{i}", [128, J], bf16))
            for i in range(NB)
        ]
        yts = [
            st.enter_context(nc.sbuf_tensor(f"yt{i}", [128, J], bf16))
            for i in range(NB)
        ]
        pss = [
            st.enter_context(nc.psum_tensor(f"ps{i}", [128, PC], f32))
            for i in range(NPS)
        ]

        def wmat(g, which):               # which: 0=prev-block, 1=cur-block
            c0 = (g * 2 + which) * 128
            return wsb[:, c0 : c0 + 128]

        with (
            nc.Block() as block,
            nc.semaphore("wsem") as wsem,
            nc.semaphore("act") as act,
            nc.semaphore("pe") as pe,
            contextlib.ExitStack() as sems,
        ):
            din = [
                sems.enter_context(nc.semaphore(f"din{i}")) for i in range(NB)
            ]
            dout = [
                sems.enter_context(nc.semaphore(f"dout{i}")) for i in range(NB)
            ]

            @block.gpsimd
            def _(gpsimd):
                # weight/bias loads ride the software DGE so SP's HWDGE ring
                # streams x uninterrupted
                gpsimd.dma_start(out=wsb[:, :], in_=w_d[:, :]).then_inc(wsem, 16)
                gpsimd.dma_start(out=zsb[:, :], in_=z_d[:, :]).then_inc(wsem, 16)

            @block.sync
            def _(sync):
                for k in range(NU):
                    if k >= NB:
                        # xt slot free once PE consumed tile k-NB
                        sync.wait_ge(pe, NH * (k - NB + 1))
                    sync.dma_start(
                        out=xts[k % NB][:, :],
                        in_=x_d[k * 128 : (k + 1) * 128, :],
                    ).then_inc(din[k % NB], 16)

            @block.tensor
            def _(tensor):
                # pe/act semaphores count PC-col chunks, NH per tile; psum
                # buffers rotate over NPS chunks
                tensor.wait_ge(wsem, 32)
                for k in range(NU):
                    tensor.wait_ge(din[k % NB], 16 * (k // NB + 1))
                    xt = xts[k % NB]
                    g = k % NG
                    for h in range(NH):
                        G = k * NH + h
                        if G >= NPS:
                            # psum buffer free once silu of chunk G-NPS done
                            tensor.wait_ge(act, G - NPS + 1)
                        ps = pss[G % NPS]
                        for c2 in range(PC // NC_CHUNK):
                            c0 = h * PC + c2 * NC_CHUNK   # within the tile
                            p0 = c2 * NC_CHUNK            # within the psum buf
                            if c0 == 0:
                                # block -1 is the causal zero block: psum col
                                # 0 gets no prev contribution. cur starts the
                                # group (zeroes the whole 512-col bank).
                                mm = tensor.matmul(
                                    ps[:, 0:NC_CHUNK],
                                    wmat(g, 1),
                                    xt[:, 0:NC_CHUNK],
                                    start=True,
                                    stop=False,
                                    skip_group_check=True,
                                )
                                mm = tensor.matmul(
                                    ps[:, 1:NC_CHUNK],
                                    wmat(g, 0),
                                    xt[:, 0 : NC_CHUNK - 1],
                                    start=False,
                                    stop=True,
                                    skip_group_check=True,
                                )
                            else:
                                mm = tensor.matmul(
                                    ps[:, p0 : p0 + NC_CHUNK],
                                    wmat(g, 0),
                                    xt[:, c0 - 1 : c0 - 1 + NC_CHUNK],
                                    start=True,
                                    stop=False,
                                    skip_group_check=True,
                                )
                                mm = tensor.matmul(
                                    ps[:, p0 : p0 + NC_CHUNK],
                                    wmat(g, 1),
                                    xt[:, c0 : c0 + NC_CHUNK],
                                    start=False,
                                    stop=True,
                                    skip_group_check=True,
                                )
                        mm.then_inc(pe)

            @block.scalar
            def _(scalar):
                func = getattr(AF, _ACT_FUNC)

                def store(k):
                    # runs while a later activation occupies the engine, so
                    # the completion inc of tile k's last chunk has already
                    # propagated: the wait is ~free and the ~900ns semaphore
                    # latency stays off the activation chain
                    scalar.wait_ge(act, NH * (k + 1))
                    scalar.dma_start(
                        out=y_d[k * 128 : (k + 1) * 128, :],
                        in_=yts[k % NB][:, :],
                    ).then_inc(dout[k % NB], 16)

                for k in range(NU):
                    for h in range(NH):
                        G = k * NH + h
                        scalar.wait_ge(pe, G + 1)
                        if h == 0 and k >= NB:
                            # yt slot's previous store (tile k-NB) done
                            scalar.wait_ge(dout[k % NB], 16 * (k // NB))
                        scalar.activation(
                            out=yts[k % NB][:, h * PC : (h + 1) * PC],
                            in_=pss[G % NPS][:, :],
                            func=func,
                            bias=0.0 if func == AF.Copy else zsb[:, :],
                            scale=1.0,
                        ).then_inc(act)
                        if h == 1 and k >= 1:
                            store(k - 1)
                store(NU - 1)
                for i in range(NB):
                    n_stores = len([k for k in range(NU) if k % NB == i])
                    scalar.wait_ge(dout[i], 16 * n_stores)

    return nc


def _weight_tables(ws):
    """ws: (HC, K) f32 -> [128, NG*2*128] bf16 banded stationary matrices.

    Partition index q = 4*rho + p. For output y[r, P*j+p_out], the cur-block
    matmul reads x[r, P*j+p_in] with tap i = p_in - p_out + 3 (p_in <= p_out);
    the prev-block matmul reads x[r, P*(j-1)+p_in] with tap i = p_in - p_out - 1
    (p_in > p_out). matmul computes out[q_out] = sum_q_in lhsT[q_in, q_out]*rhs.
    """
    wtab = np.zeros((128, NG * 2 * 128), np.float32)
    rho = np.arange(RPU)
    for g in range(NG):
        ch = ws[RPU * g : RPU * (g + 1)]          # (RPU, K)
        Wp = np.zeros((128, 128), np.float32)
        Wc = np.zeros((128, 128), np.float32)
        for p_in in range(P):
            for p_out in range(P):
                i = p_in - p_out + (K - 1)
                if 0 <= i < K:
                    Wc[P * rho + p_in, P * rho + p_out] = ch[:, i]
                i = p_in - p_out - 1
                if 0 <= i < K:
                    Wp[P * rho + p_in, P * rho + p_out] = ch[:, i]
        wtab[:, (2 * g) * 128 : (2 * g + 1) * 128] = Wp
        wtab[:, (2 * g + 1) * 128 : (2 * g + 2) * 128] = Wc
    return wtab.astype(BF16)


def kernel(x, weight):
    global _last_results
    from concourse.bass_utils import run_bass_kernel_spmd

    x = np.asarray(x, dtype=np.float32)
    weight = np.asarray(weight, dtype=np.float32)

    nc = _build_program()

    zeros = np.zeros((128, 1), np.float32)
    in_maps = []
    for core in range(N_CORES):
        sl = slice(core * HC, (core + 1) * HC)
        # [B, S, HC] -> [B, HC, S] -> [ROWS, S], row r = b*HC + c
        xc = x[:, :, sl].transpose(0, 2, 1).reshape(ROWS, S)
        # phase split: row 4r+p, col j = x[r, 4j+p]
        xs = np.ascontiguousarray(
            xc.reshape(ROWS, J, P).transpose(0, 2, 1).reshape(ROWS * P, J)
        ).astype(BF16)
        in_maps.append({"x": xs, "w": _weight_tables(weight[sl, :]), "z": zeros})

    res = run_bass_kernel_spmd(nc, in_maps, list(range(N_CORES)))
    _last_results = res

    out = np.empty((B, S, H), np.float32)
    for core in range(N_CORES):
        sl = slice(core * HC, (core + 1) * HC)
        yc = np.asarray(res.results[core]["y"], dtype=np.float32)
        # undo phase split, then row-major [B, HC, S] -> [B, S, HC]
        yc = yc.reshape(ROWS, P, J).transpose(0, 2, 1).reshape(B, HC, S)
        out[:, :, sl] = yc.transpose(0, 2, 1)
    return out


# revision 19
# speedup vs baseline: 2.0220x; 1.0822x over previous
"""Causal depthwise conv1d (K=4) + SiLU, sharded over 8 NeuronCores.

Full shapes: x [4, 8192, 2048] f32, weight [2048, 4] f32 -> y [4, 8192, 2048] f32.

Strategy: tensor-parallel over the hidden/channel dim (fully channel
independent, no halo exchange). Each core gets 256 channels -> 1024
independent rows (batch x channel). All HBM traffic is bf16 (the 2e-2
rel-err budget dwarfs bf16's ~1e-3), halving the memory-bound roofline
vs f32.

Layout: time is phase-split host-side, t = 4j + p. SBUF partition dim
packs (32 rows x 4 phases); the free dim is the block index j. A causal
conv tap then only ever reads the current block j or block j-1, so each
512-block PSUM chunk needs just TWO 128x128 banded-matmul accumulations
(prev-block taps + cur-block taps) instead of one diag matmul per tap:
2x less TensorEngine time than the diagonal formulation, keeping PE
(~55us) under the bf16 DMA roofline (~94us). The banded weight matrices
(block-diagonal over rows, 4x4 tap bands over phases) are built host-side
and DMA'd once. A leading zero block column provides causal padding.

Compute: PE accumulates bf16 matmuls into f32 PSUM; ACT applies SiLU
straight out of PSUM, writing bf16. Output DMAs are triggered by the
otherwise-idle DVE so the ~900ns semaphore-propagation wait between an
activation and its store never blocks the next activation; inputs stream
on SP's ring. x is stored unpadded (4096B-aligned partition lines — the
odd 2-byte halo column measurably degraded input DMA bandwidth); the
causal zero block is handled by skipping the prev-block matmul's first
output column in chunk 0.

Raw bass (no Tile framework): the installed walrus codegen only accepts one
sync wait per compute instruction, so all synchronization is explicit wait_ge
sequencer instructions. Per-buffer-slot DMA semaphores keep concurrent DMA
completion increments unambiguous. Sem increments fire at instruction
completion, but the sequencer runs ahead, so consumers of an engine's result
always gate on that completion increment (including same-engine self-waits
before DMA triggers).
"""

import contextlib

import numpy as np
import ml_dtypes

B, S, H, K = 4, 8192, 2048, 4
N_CORES = 8
HC = H // N_CORES          # 256 channels per core
ROWS = B * HC              # 1024 rows per core, row r = b*HC + c
P = 4                      # time phases per partition group, t = P*j + p
J = S // P                 # 2048 blocks
RPU = 128 // P             # 32 rows per partition unit
NU = ROWS // RPU           # 32 units (tiles); tile k = unit k, all blocks
NG = HC // RPU             # 8 distinct weight groups (weights repeat per b)
NB = 8                     # buffers per tile kind
NC_CHUNK = 512             # one PSUM bank of fp32
NCHUNKS = J // NC_CHUNK    # 4
PC = 1024                  # psum buffer / activation chunk (2 banks)
NH = J // PC               # 2 chunks per tile
NPS = 4                    # psum buffers (all 8 banks); ping depth 4

BF16 = ml_dtypes.bfloat16

_last_results = None       # test harness introspection (exec_time_ns etc.)
_ACT_FUNC = "Silu"         # sim override hook (CoreSim lacks Silu)


def _build_program():
    from concourse import bass, mybir

    f32 = mybir.dt.float32
    bf16 = mybir.dt.bfloat16
    AF = mybir.ActivationFunctionType

    nc = bass.Bass()
    # phase-split x: row 128*u + 4*rho + p holds x[32u+rho, P*j+p] at col j
    x_d = nc.declare_dram_parameter("x", [NU * 128, J], bf16, isOutput=False)
    # 16 stationary matrices: [Wprev_g | Wcur_g] for g in 0..NG
    w_d = nc.declare_dram_parameter("w", [128, NG * 2 * 128], bf16, isOutput=False)
    z_d = nc.declare_dram_parameter("z", [128, 1], f32, isOutput=False)
    y_d = nc.declare_dram_parameter("y", [NU * 128, J], bf16, isOutput=True)

    with contextlib.ExitStack() as st:
        wsb = st.enter_context(nc.sbuf_tensor("wsb", [128, NG * 2 * 128], bf16))
        zsb = st.enter_context(nc.sbuf_tensor("zsb", [128, 1], f32))
        xts = [
            st.enter_context(nc.sbuf_tensor(f"xt{i}", [128, J], bf16))
            for i in range(NB)
        ]
        yts = [
            st.enter_context(nc.sbuf_tensor(f"yt{i}", [128, J], bf16))
            for i in range(NB)
        ]
        pss = [
            st.enter_context(nc.psum_tensor(f"ps{i}", [128, PC], f32))
            for i in range(NPS)
        ]

        def wmat(g, which):               # which: 0=prev-block, 1=cur-block
            c0 = (g * 2 + which) * 128
            return wsb[:, c0 : c0 + 128]

        with (
            nc.Block() as block,
            nc.semaphore("wsem") as wsem,
            nc.semaphore("act") as act,
            nc.semaphore("pe") as pe,
            nc.semaphore("dl") as dl,
            contextlib.ExitStack() as sems,
        ):
            din = [
                sems.enter_context(nc.semaphore(f"din{i}")) for i in range(NB)
            ]
            dout = [
                sems.enter_context(nc.semaphore(f"dout{i}")) for i in range(NB)
            ]

            @block.gpsimd
            def _(gpsimd):
                # weight/bias loads ride the software DGE so SP's HWDGE ring
                # streams x uninterrupted
                gpsimd.dma_start(out=wsb[:, :], in_=w_d[:, :]).then_inc(wsem, 16)
                gpsimd.dma_start(out=zsb[:, :], in_=z_d[:, :]).then_inc(wsem, 16)

            @block.sync
            def _(sync):
                for k in range(NU):
                    if k >= NB:
                        # xt slot free once PE consumed tile k-NB
                        sync.wait_ge(pe, NH * (k - NB + 1))
                    if k == NU - 1:
                        # split the last tile's load per PC-chunk so the tail
                        # PE/ACT/store pipeline starts before the full tile
                        # lands. The halves complete out of order across the
                        # DMA engines, so half B gets its own semaphore —
                        # mixing both halves' incs on din would let PE start
                        # chunk 0 before half A fully landed.
                        for h, sem in ((0, din[k % NB]), (1, dl)):
                            sync.dma_start(
                                out=xts[k % NB][:, h * PC : (h + 1) * PC],
                                in_=x_d[k * 128 : (k + 1) * 128, h * PC : (h + 1) * PC],
                            ).then_inc(sem, 16)
                    else:
                        sync.dma_start(
                            out=xts[k % NB][:, :],
                            in_=x_d[k * 128 : (k + 1) * 128, :],
                        ).then_inc(din[k % NB], 16)

            @block.tensor
            def _(tensor):
                # pe/act semaphores count PC-col chunks, NH per tile; psum
                # buffers rotate over NPS chunks
                tensor.wait_ge(wsem, 32)
                for k in range(NU):
                    if k < NU - 1:
                        tensor.wait_ge(din[k % NB], 16 * (k // NB + 1))
                    xt = xts[k % NB]
                    g = k % NG
                    for h in range(NH):
                        G = k * NH + h
                        if k == NU - 1:
                            # split load: chunk 0 needs half A, chunk 1 both
                            if h == 0:
                                tensor.wait_ge(din[k % NB], 16 * (k // NB + 1))
                            else:
                                tensor.wait_ge(dl, 16)
                        if G >= NPS:
                            # psum buffer free once silu of chunk G-NPS done
                            tensor.wait_ge(act, G - NPS + 1)
                        ps = pss[G % NPS]
                        for c2 in range(PC // NC_CHUNK):
                            c0 = h * PC + c2 * NC_CHUNK   # within the tile
                            p0 = c2 * NC_CHUNK            # within the psum buf
                            if c0 == 0:
                                # block -1 is the causal zero block: psum col
                                # 0 gets no prev contribution. cur starts the
                                # group (zeroes the whole 512-col bank).
                                mm = tensor.matmul(
                                    ps[:, 0:NC_CHUNK],
                                    wmat(g, 1),
                                    xt[:, 0:NC_CHUNK],
                                    start=True,
                                    stop=False,
                                    skip_group_check=True,
                                )
                                mm = tensor.matmul(
                                    ps[:, 1:NC_CHUNK],
                                    wmat(g, 0),
                                    xt[:, 0 : NC_CHUNK - 1],
                                    start=False,
                                    stop=True,
                                    skip_group_check=True,
                                )
                            else:
                                mm = tensor.matmul(
                                    ps[:, p0 : p0 + NC_CHUNK],
                                    wmat(g, 0),
                                    xt[:, c0 - 1 : c0 - 1 + NC_CHUNK],
                                    start=True,
                                    stop=False,
                                    skip_group_check=True,
                                )
                                mm = tensor.matmul(
                                    ps[:, p0 : p0 + NC_CHUNK],
                                    wmat(g, 1),
                                    xt[:, c0 : c0 + NC_CHUNK],
                                    start=False,
                                    stop=True,
                                    skip_group_check=True,
                                )
                        mm.then_inc(pe)

            @block.scalar
            def _(scalar):
                func = getattr(AF, _ACT_FUNC)

                def store(k):
                    # runs while a later activation occupies the engine, so
                    # the completion inc of tile k's last chunk has already
                    # propagated: the wait is ~free and the ~900ns semaphore
                    # latency stays off the activation chain
                    scalar.wait_ge(act, NH * (k + 1))
                    scalar.dma_start(
                        out=y_d[k * 128 : (k + 1) * 128, :],
                        in_=yts[k % NB][:, :],
                    ).then_inc(dout[k % NB], 16)

                for k in range(NU):
                    for h in range(NH):
                        G = k * NH + h
                        scalar.wait_ge(pe, G + 1)
                        if h == 0 and k >= NB:
                            # yt slot's previous store (tile k-NB) done
                            scalar.wait_ge(dout[k % NB], 16 * (k // NB))
                        scalar.activation(
                            out=yts[k % NB][:, h * PC : (h + 1) * PC],
                            in_=pss[G % NPS][:, :],
                            func=func,
                            bias=0.0 if func == AF.Copy else zsb[:, :],
                            scale=1.0,
                        ).then_inc(act)
                        if h == 1 and k >= 1:
                            store(k - 1)
                # tail: store the last tile per chunk so the final store
                # overlaps the final activation instead of trailing it
                kl = NU - 1
                for h in range(NH):
                    scalar.wait_ge(act, NH * kl + h + 1)
                    scalar.dma_start(
                        out=y_d[kl * 128 : (kl + 1) * 128, h * PC : (h + 1) * PC],
                        in_=yts[kl % NB][:, h * PC : (h + 1) * PC],
                    ).then_inc(dout[kl % NB], 16)
                for i in range(NB):
                    n_stores = len(
                        [k for k in range(NU - 1) if k % NB == i]
                    ) + (NH if i == kl % NB else 0)
                    scalar.wait_ge(dout[i], 16 * n_stores)

    return nc


def _weight_tables(ws):
    """ws: (HC, K) f32 -> [128, NG*2*128] bf16 banded stationary matrices.

    Partition index q = 4*rho + p. For output y[r, P*j+p_out], the cur-block
    matmul reads x[r, P*j+p_in] with tap i = p_in - p_out + 3 (p_in <= p_out);
    the prev-block matmul reads x[r, P*(j-1)+p_in] with tap i = p_in - p_out - 1
    (p_in > p_out). matmul computes out[q_out] = sum_q_in lhsT[q_in, q_out]*rhs.
    """
    wtab = np.zeros((128, NG * 2 * 128), np.float32)
    rho = np.arange(RPU)
    for g in range(NG):
        ch = ws[RPU * g : RPU * (g + 1)]          # (RPU, K)
        Wp = np.zeros((128, 128), np.float32)
        Wc = np.zeros((128, 128), np.float32)
        for p_in in range(P):
            for p_out in range(P):
                i = p_in - p_out + (K - 1)
                if 0 <= i < K:
                    Wc[P * rho + p_in, P * rho + p_out] = ch[:, i]
                i = p_in - p_out - 1
                if 0 <= i < K:
                    Wp[P * rho + p_in, P * rho + p_out] = ch[:, i]
        wtab[:, (2 * g) * 128 : (2 * g + 1) * 128] = Wp
        wtab[:, (2 * g + 1) * 128 : (2 * g + 2) * 128] = Wc
    return wtab.astype(BF16)


def kernel(x, weight):
    global _last_results
    from concourse.bass_utils import run_bass_kernel_spmd

    x = np.asarray(x, dtype=np.float32)
    weight = np.asarray(weight, dtype=np.float32)

    nc = _build_program()

    zeros = np.zeros((128, 1), np.float32)
    in_maps = []
    for core in range(N_CORES):
        sl = slice(core * HC, (core + 1) * HC)
        # [B, S, HC] -> [B, HC, S] -> [ROWS, S], row r = b*HC + c
        xc = x[:, :, sl].transpose(0, 2, 1).reshape(ROWS, S)
        # phase split: row 4r+p, col j = x[r, 4j+p]
        xs = np.ascontiguousarray(
            xc.reshape(ROWS, J, P).transpose(0, 2, 1).reshape(ROWS * P, J)
        ).astype(BF16)
        in_maps.append({"x": xs, "w": _weight_tables(weight[sl, :]), "z": zeros})

    res = run_bass_kernel_spmd(nc, in_maps, list(range(N_CORES)))
    _last_results = res

    out = np.empty((B, S, H), np.float32)
    for core in range(N_CORES):
        sl = slice(core * HC, (core + 1) * HC)
        yc = np.asarray(res.results[core]["y"], dtype=np.float32)
        # undo phase split, then row-major [B, HC, S] -> [B, S, HC]
        yc = yc.reshape(ROWS, P, J).transpose(0, 2, 1).reshape(B, HC, S)
        out[:, :, sl] = yc.transpose(0, 2, 1)
    return out


# revision 20
# speedup vs baseline: 2.0306x; 1.0042x over previous
"""Causal depthwise conv1d (K=4) + SiLU, sharded over 8 NeuronCores.

Full shapes: x [4, 8192, 2048] f32, weight [2048, 4] f32 -> y [4, 8192, 2048] f32.

Strategy: tensor-parallel over the hidden/channel dim (fully channel
independent, no halo exchange). Each core gets 256 channels -> 1024
independent rows (batch x channel). All HBM traffic is bf16 (the 2e-2
rel-err budget dwarfs bf16's ~1e-3), halving the memory-bound roofline
vs f32.

Layout: time is phase-split host-side, t = 4j + p. SBUF partition dim
packs (32 rows x 4 phases); the free dim is the block index j. A causal
conv tap then only ever reads the current block j or block j-1, so each
512-block PSUM chunk needs just TWO 128x128 banded-matmul accumulations
(prev-block taps + cur-block taps) instead of one diag matmul per tap:
2x less TensorEngine time than the diagonal formulation, keeping PE
(~55us) under the bf16 DMA roofline (~94us). The banded weight matrices
(block-diagonal over rows, 4x4 tap bands over phases) are built host-side
and DMA'd once. A leading zero block column provides causal padding.

Compute: PE accumulates bf16 matmuls into f32 PSUM; ACT applies SiLU
straight out of PSUM, writing bf16. Output DMAs are triggered by the
otherwise-idle DVE so the ~900ns semaphore-propagation wait between an
activation and its store never blocks the next activation; inputs stream
on SP's ring. x is stored unpadded (4096B-aligned partition lines — the
odd 2-byte halo column measurably degraded input DMA bandwidth); the
causal zero block is handled by skipping the prev-block matmul's first
output column in chunk 0.

Raw bass (no Tile framework): the installed walrus codegen only accepts one
sync wait per compute instruction, so all synchronization is explicit wait_ge
sequencer instructions. Per-buffer-slot DMA semaphores keep concurrent DMA
completion increments unambiguous. Sem increments fire at instruction
completion, but the sequencer runs ahead, so consumers of an engine's result
always gate on that completion increment (including same-engine self-waits
before DMA triggers).
"""

import contextlib

import numpy as np
import ml_dtypes

B, S, H, K = 4, 8192, 2048, 4
N_CORES = 8
HC = H // N_CORES          # 256 channels per core
ROWS = B * HC              # 1024 rows per core, row r = b*HC + c
P = 4                      # time phases per partition group, t = P*j + p
J = S // P                 # 2048 blocks
RPU = 128 // P             # 32 rows per partition unit
NU = ROWS // RPU           # 32 units (tiles); tile k = unit k, all blocks
NG = HC // RPU             # 8 distinct weight groups (weights repeat per b)
NB = 8                     # buffers per tile kind
NC_CHUNK = 512             # one PSUM bank of fp32
NCHUNKS = J // NC_CHUNK    # 4
PC = 1024                  # psum buffer / activation chunk (2 banks)
NH = J // PC               # 2 chunks per tile
NPS = 4                    # psum buffers (all 8 banks); ping depth 4

BF16 = ml_dtypes.bfloat16

_last_results = None       # test harness introspection (exec_time_ns etc.)
_ACT_FUNC = "Silu"         # sim override hook (CoreSim lacks Silu)


def _build_program():
    from concourse import bass, mybir

    f32 = mybir.dt.float32
    bf16 = mybir.dt.bfloat16
    AF = mybir.ActivationFunctionType

    nc = bass.Bass()
    # phase-split x: row 128*u + 4*rho + p holds x[32u+rho, P*j+p] at col j
    x_d = nc.declare_dram_parameter("x", [NU * 128, J], bf16, isOutput=False)
    # 16 stationary matrices: [Wprev_g | Wcur_g] for g in 0..NG
    w_d = nc.declare_dram_parameter("w", [128, NG * 2 * 128], bf16, isOutput=False)
    z_d = nc.declare_dram_parameter("z", [128, 1], f32, isOutput=False)
    y_d = nc.declare_dram_parameter("y", [NU * 128, J], bf16, isOutput=True)

    with contextlib.ExitStack() as st:
        wsb = st.enter_context(nc.sbuf_tensor("wsb", [128, NG * 2 * 128], bf16))
        zsb = st.enter_context(nc.sbuf_tensor("zsb", [128, 1], f32))
        xts = [
            st.enter_context(nc.sbuf_tensor(f"xt{i}", [128, J], bf16))
            for i in range(NB)
        ]
        yts = [
            st.enter_context(nc.sbuf_tensor(f"yt{i}", [128, J], bf16))
            for i in range(NB)
        ]
        pss = [
            st.enter_context(nc.psum_tensor(f"ps{i}", [128, PC], f32))
            for i in range(NPS)
        ]

        def wmat(g, which):               # which: 0=prev-block, 1=cur-block
            c0 = (g * 2 + which) * 128
            return wsb[:, c0 : c0 + 128]

        with (
            nc.Block() as block,
            nc.semaphore("wsem") as wsem,
            nc.semaphore("act") as act,
            nc.semaphore("pe") as pe,
            nc.semaphore("dl") as dl,
            contextlib.ExitStack() as sems,
        ):
            din = [
                sems.enter_context(nc.semaphore(f"din{i}")) for i in range(NB)
            ]
            dout = [
                sems.enter_context(nc.semaphore(f"dout{i}")) for i in range(NB)
            ]

            @block.gpsimd
            def _(gpsimd):
                # weight/bias loads ride the software DGE so SP's HWDGE ring
                # streams x uninterrupted
                gpsimd.dma_start(out=wsb[:, :], in_=w_d[:, :]).then_inc(wsem, 16)
                gpsimd.dma_start(out=zsb[:, :], in_=z_d[:, :]).then_inc(wsem, 16)

            @block.sync
            def _(sync):
                for k in range(NU):
                    if k >= NB:
                        # xt slot free once PE consumed tile k-NB
                        sync.wait_ge(pe, NH * (k - NB + 1))
                    if k == NU - 1:
                        # split the last tile's load per PC-chunk so the tail
                        # PE/ACT/store pipeline starts before the full tile
                        # lands. The halves complete out of order across the
                        # DMA engines, so half B gets its own semaphore —
                        # mixing both halves' incs on din would let PE start
                        # chunk 0 before half A fully landed.
                        for h, sem in ((0, din[k % NB]), (1, dl)):
                            sync.dma_start(
                                out=xts[k % NB][:, h * PC : (h + 1) * PC],
                                in_=x_d[k * 128 : (k + 1) * 128, h * PC : (h + 1) * PC],
                            ).then_inc(sem, 16)
                    else:
                        sync.dma_start(
                            out=xts[k % NB][:, :],
                            in_=x_d[k * 128 : (k + 1) * 128, :],
                        ).then_inc(din[k % NB], 16)

            @block.tensor
            def _(tensor):
                # pe/act semaphores count PC-col chunks, NH per tile; psum
                # buffers rotate over NPS chunks
                tensor.wait_ge(wsem, 32)
                for k in range(NU):
                    if k < NU - 1:
                        tensor.wait_ge(din[k % NB], 16 * (k // NB + 1))
                    xt = xts[k % NB]
                    g = k % NG
                    for h in range(NH):
                        G = k * NH + h
                        if k == NU - 1:
                            # split load: chunk 0 needs half A, chunk 1 both
                            if h == 0:
                                tensor.wait_ge(din[k % NB], 16 * (k // NB + 1))
                            else:
                                tensor.wait_ge(dl, 16)
                        if G >= NPS:
                            # psum buffer free once silu of chunk G-NPS done
                            tensor.wait_ge(act, G - NPS + 1)
                        ps = pss[G % NPS]
                        for c2 in range(PC // NC_CHUNK):
                            c0 = h * PC + c2 * NC_CHUNK   # within the tile
                            p0 = c2 * NC_CHUNK            # within the psum buf
                            if c0 == 0:
                                # block -1 is the causal zero block: psum col
                                # 0 gets no prev contribution. cur starts the
                                # group (zeroes the whole 512-col bank).
                                mm = tensor.matmul(
                                    ps[:, 0:NC_CHUNK],
                                    wmat(g, 1),
                                    xt[:, 0:NC_CHUNK],
                                    start=True,
                                    stop=False,
                                    skip_group_check=True,
                                )
                                mm = tensor.matmul(
                                    ps[:, 1:NC_CHUNK],
                                    wmat(g, 0),
                                    xt[:, 0 : NC_CHUNK - 1],
                                    start=False,
                                    stop=True,
                                    skip_group_check=True,
                                )
                            else:
                                mm = tensor.matmul(
                                    ps[:, p0 : p0 + NC_CHUNK],
                                    wmat(g, 0),
                                    xt[:, c0 - 1 : c0 - 1 + NC_CHUNK],
                                    start=True,
                                    stop=False,
                                    skip_group_check=True,
                                )
                                mm = tensor.matmul(
                                    ps[:, p0 : p0 + NC_CHUNK],
                                    wmat(g, 1),
                                    xt[:, c0 : c0 + NC_CHUNK],
                                    start=False,
                                    stop=True,
                                    skip_group_check=True,
                                )
                        mm.then_inc(pe)

            @block.scalar
            def _(scalar):
                func = getattr(AF, _ACT_FUNC)

                def store_chunk(G):
                    # runs while the next activation occupies the engine, so
                    # chunk G's completion inc has already propagated: the
                    # wait is ~free and the ~900ns semaphore latency stays
                    # off the activation chain. Chunk-granular stores keep
                    # the output queue only ~1 chunk behind the data.
                    k, h = G // NH, G % NH
                    scalar.wait_ge(act, G + 1)
                    scalar.dma_start(
                        out=y_d[k * 128 : (k + 1) * 128, h * PC : (h + 1) * PC],
                        in_=yts[k % NB][:, h * PC : (h + 1) * PC],
                    ).then_inc(dout[k % NB], 16)

                for k in range(NU):
                    for h in range(NH):
                        G = k * NH + h
                        scalar.wait_ge(pe, G + 1)
                        if h == 0 and k >= NB:
                            # yt slot's previous stores (tile k-NB) done;
                            # total-count gate, so the two chunks' incs
                            # mixing on one semaphore is unambiguous
                            scalar.wait_ge(dout[k % NB], 16 * NH * (k // NB))
                        scalar.activation(
                            out=yts[k % NB][:, h * PC : (h + 1) * PC],
                            in_=pss[G % NPS][:, :],
                            func=func,
                            bias=0.0 if func == AF.Copy else zsb[:, :],
                            scale=1.0,
                        ).then_inc(act)
                        if G >= 1:
                            store_chunk(G - 1)
                store_chunk(NU * NH - 1)
                for i in range(NB):
                    n_tiles = len([k for k in range(NU) if k % NB == i])
                    scalar.wait_ge(dout[i], 16 * NH * n_tiles)

    return nc


def _weight_tables(ws):
    """ws: (HC, K) f32 -> [128, NG*2*128] bf16 banded stationary matrices.

    Partition index q = 4*rho + p. For output y[r, P*j+p_out], the cur-block
    matmul reads x[r, P*j+p_in] with tap i = p_in - p_out + 3 (p_in <= p_out);
    the prev-block matmul reads x[r, P*(j-1)+p_in] with tap i = p_in - p_out - 1
    (p_in > p_out). matmul computes out[q_out] = sum_q_in lhsT[q_in, q_out]*rhs.
    """
    wtab = np.zeros((128, NG * 2 * 128), np.float32)
    rho = np.arange(RPU)
    for g in range(NG):
        ch = ws[RPU * g : RPU * (g + 1)]          # (RPU, K)
        Wp = np.zeros((128, 128), np.float32)
        Wc = np.zeros((128, 128), np.float32)
        for p_in in range(P):
            for p_out in range(P):
                i = p_in - p_out + (K - 1)
                if 0 <= i < K:
                    Wc[P * rho + p_in, P * rho + p_out] = ch[:, i]
                i = p_in - p_out - 1
                if 0 <= i < K:
                    Wp[P * rho + p_in, P * rho + p_out] = ch[:, i]
        wtab[:, (2 * g) * 128 : (2 * g + 1) * 128] = Wp
        wtab[:, (2 * g + 1) * 128 : (2 * g + 2) * 128] = Wc
    return wtab.astype(BF16)


def kernel(x, weight):
    global _last_results
    from concourse.bass_utils import run_bass_kernel_spmd

    x = np.asarray(x, dtype=np.float32)
    weight = np.asarray(weight, dtype=np.float32)

    nc = _build_program()

    zeros = np.zeros((128, 1), np.float32)
    in_maps = []
    for core in range(N_CORES):
        sl = slice(core * HC, (core + 1) * HC)
        # [B, S, HC] -> [B, HC, S] -> [ROWS, S], row r = b*HC + c
        xc = x[:, :, sl].transpose(0, 2, 1).reshape(ROWS, S)
        # phase split: row 4r+p, col j = x[r, 4j+p]
        xs = np.ascontiguousarray(
            xc.reshape(ROWS, J, P).transpose(0, 2, 1).reshape(ROWS * P, J)
        ).astype(BF16)
        in_maps.append({"x": xs, "w": _weight_tables(weight[sl, :]), "z": zeros})

    res = run_bass_kernel_spmd(nc, in_maps, list(range(N_CORES)))
    _last_results = res

    out = np.empty((B, S, H), np.float32)
    for core in range(N_CORES):
        sl = slice(core * HC, (core + 1) * HC)
        yc = np.asarray(res.results[core]["y"], dtype=np.float32)
        # undo phase split, then row-major [B, HC, S] -> [B, S, HC]
        yc = yc.reshape(ROWS, P, J).transpose(0, 2, 1).reshape(B, HC, S)
        out[:, :, sl] = yc.transpose(0, 2, 1)
    return out


# revision 27
# speedup vs baseline: 2.0442x; 1.0067x over previous
"""Causal depthwise conv1d (K=4) + SiLU, sharded over 8 NeuronCores.

Full shapes: x [4, 8192, 2048] f32, weight [2048, 4] f32 -> y [4, 8192, 2048] f32.

Strategy: tensor-parallel over the hidden/channel dim (fully channel
independent, no halo exchange). Each core gets 256 channels -> 1024
independent rows (batch x channel). All HBM traffic is bf16 (the 2e-2
rel-err budget dwarfs bf16's ~1e-3), halving the memory-bound roofline
vs f32.

Layout: time is phase-split host-side, t = 4j + p. SBUF partition dim
packs (32 rows x 4 phases); the free dim is the block index j. A causal
conv tap then only ever reads the current block j or block j-1, so each
512-block PSUM chunk needs just TWO 128x128 banded-matmul accumulations
(prev-block taps + cur-block taps) instead of one diag matmul per tap:
2x less TensorEngine time than the diagonal formulation, keeping PE
(~55us) under the bf16 DMA roofline (~94us). The banded weight matrices
(block-diagonal over rows, 4x4 tap bands over phases) are built host-side
and DMA'd once. A leading zero block column provides causal padding.

Compute: PE accumulates bf16 matmuls into f32 PSUM; ACT applies SiLU
straight out of PSUM, writing bf16. Output DMAs are triggered by the
otherwise-idle DVE so the ~900ns semaphore-propagation wait between an
activation and its store never blocks the next activation; inputs stream
on SP's ring. x is stored unpadded (4096B-aligned partition lines — the
odd 2-byte halo column measurably degraded input DMA bandwidth); the
causal zero block is handled by skipping the prev-block matmul's first
output column in chunk 0.

Raw bass (no Tile framework): the installed walrus codegen only accepts one
sync wait per compute instruction, so all synchronization is explicit wait_ge
sequencer instructions. Per-buffer-slot DMA semaphores keep concurrent DMA
completion increments unambiguous. Sem increments fire at instruction
completion, but the sequencer runs ahead, so consumers of an engine's result
always gate on that completion increment (including same-engine self-waits
before DMA triggers).
"""

import contextlib

import numpy as np
import ml_dtypes

B, S, H, K = 4, 8192, 2048, 4
N_CORES = 8
HC = H // N_CORES          # 256 channels per core
ROWS = B * HC              # 1024 rows per core, row r = b*HC + c
P = 4                      # time phases per partition group, t = P*j + p
J = S // P                 # 2048 blocks
RPU = 128 // P             # 32 rows per partition unit
NU = ROWS // RPU           # 32 units (tiles); tile k = unit k, all blocks
NG = HC // RPU             # 8 distinct weight groups (weights repeat per b)
NB = 8                     # buffers per tile kind
NC_CHUNK = 512             # one PSUM bank of fp32
NCHUNKS = J // NC_CHUNK    # 4
PC = 1024                  # psum buffer / activation chunk (2 banks)
NH = J // PC               # 2 chunks per tile
NPS = 4                    # psum buffers (all 8 banks); ping depth 4

BF16 = ml_dtypes.bfloat16

_last_results = None       # test harness introspection (exec_time_ns etc.)
_ACT_FUNC = "Silu"         # sim override hook (CoreSim lacks Silu)


def _build_program():
    from concourse import bass, mybir

    f32 = mybir.dt.float32
    bf16 = mybir.dt.bfloat16
    AF = mybir.ActivationFunctionType

    nc = bass.Bass()
    # phase-split x: row 128*u + 4*rho + p holds x[32u+rho, P*j+p] at col j
    x_d = nc.declare_dram_parameter("x", [NU * 128, J], bf16, isOutput=False)
    # compact per-diagonal scalars (cur: NG*K cols, prev: NG*(K-1) cols,
    # last col zeros for the Silu bias); the dense banded stationaries are
    # assembled on-device by the otherwise-idle GpSimd+DVE to keep 512KB of
    # weight-table DMA off the HBM-saturated stream
    SCC = NG * K + NG * (K - 1) + 1
    sc_d = nc.declare_dram_parameter("sc", [128, SCC], f32, isOutput=False)
    y_d = nc.declare_dram_parameter("y", [NU * 128, J], bf16, isOutput=True)

    with contextlib.ExitStack() as st:
        wsb = st.enter_context(nc.sbuf_tensor("wsb", [128, NG * 2 * 128], bf16))
        scsb = st.enter_context(nc.sbuf_tensor("scsb", [128, SCC], f32))
        ones = st.enter_context(nc.sbuf_tensor("ones", [128, 128], bf16))
        msk = st.enter_context(nc.sbuf_tensor("msk", [128, 7 * 128], bf16))
        tmp = st.enter_context(nc.sbuf_tensor("tmp", [128, 128], bf16))
        xts = [
            st.enter_context(nc.sbuf_tensor(f"xt{i}", [128, J], bf16))
            for i in range(NB)
        ]
        yts = [
            st.enter_context(nc.sbuf_tensor(f"yt{i}", [128, J], bf16))
            for i in range(NB)
        ]
        pss = [
            st.enter_context(nc.psum_tensor(f"ps{i}", [128, PC], f32))
            for i in range(NPS)
        ]

        def wmat(g, which):               # which: 0=prev-block, 1=cur-block
            c0 = (g * 2 + which) * 128
            return wsb[:, c0 : c0 + 128]

        def mdiag(d):                     # shifted-diag mask, delta = d - 3
            return msk[:, d * 128 : (d + 1) * 128]

        with (
            nc.Block() as block,
            nc.semaphore("wsem") as wsem,
            nc.semaphore("act") as act,
            nc.semaphore("pe") as pe,
            nc.semaphore("dl") as dl,
            nc.semaphore("esem") as esem,
            nc.semaphore("dve") as dve,
            contextlib.ExitStack() as sems,
        ):
            din = [
                sems.enter_context(nc.semaphore(f"din{i}")) for i in range(NB)
            ]
            dout = [
                sems.enter_context(nc.semaphore(f"dout{i}")) for i in range(NB)
            ]

            @block.gpsimd
            def _(gpsimd):
                # shifted-diagonal masks for the on-device weight build
                # (affine_select only exists on gpsimd)
                gpsimd.memset(ones[:, :], 1.0)
                for d in range(7):
                    delta = d - 3
                    gpsimd.affine_select(
                        out=mdiag(d), in_=ones[:, :],
                        pattern=[[1, 128]], base=-delta, channel_multiplier=-1,
                        compare_op=mybir.AluOpType.is_equal, fill=0.0,
                    ).then_inc(esem)

            @block.vector
            def _(vector):
                # assemble the 16 banded stationaries: each is a sum of
                # masked shifted diagonals scaled by a per-partition column
                vector.wait_ge(wsem, 16)
                vector.wait_ge(esem, 7)
                for g in range(NG):
                    wc, wp = wmat(g, 1), wmat(g, 0)
                    # cur-block: delta = 0..3, tap K-1-delta
                    vector.tensor_scalar_mul(
                        wc, mdiag(3), scsb[:, g * K : g * K + 1]
                    )
                    for delta in range(1, K):
                        vector.tensor_scalar_mul(
                            tmp[:, :], mdiag(3 + delta),
                            scsb[:, g * K + delta : g * K + delta + 1],
                        )
                        vector.tensor_add(wc, wc, tmp[:, :])
                    # prev-block: delta = -1..-3, tap -delta-1
                    c0 = NG * K + g * (K - 1)
                    vector.tensor_scalar_mul(
                        wp, mdiag(2), scsb[:, c0 : c0 + 1]
                    )
                    for dp in range(2, K):
                        vector.tensor_scalar_mul(
                            tmp[:, :], mdiag(3 - dp),
                            scsb[:, c0 + dp - 1 : c0 + dp],
                        )
                        mm = vector.tensor_add(wp, wp, tmp[:, :])
                    mm.then_inc(dve)

            @block.sync
            def _(sync):
                sync.dma_start(out=scsb[:, :], in_=sc_d[:, :]).then_inc(wsem, 16)
                for k in range(NU):
                    if k >= NB:
                        # xt slot free once PE consumed tile k-NB
                        sync.wait_ge(pe, NH * (k - NB + 1))
                    if k == NU - 1:
                        # split the last tile's load per PC-chunk so the tail
                        # PE/ACT/store pipeline starts before the full tile
                        # lands. The halves complete out of order across the
                        # DMA engines, so half B gets its own semaphore —
                        # mixing both halves' incs on din would let PE start
                        # chunk 0 before half A fully landed.
                        for h, sem in ((0, din[k % NB]), (1, dl)):
                            sync.dma_start(
                                out=xts[k % NB][:, h * PC : (h + 1) * PC],
                                in_=x_d[k * 128 : (k + 1) * 128, h * PC : (h + 1) * PC],
                            ).then_inc(sem, 16)
                    else:
                        sync.dma_start(
                            out=xts[k % NB][:, :],
                            in_=x_d[k * 128 : (k + 1) * 128, :],
                        ).then_inc(din[k % NB], 16)

            @block.tensor
            def _(tensor):
                # pe/act semaphores count PC-col chunks, NH per tile; psum
                # buffers rotate over NPS chunks
                for k in range(NU):
                    if k < NG:
                        # stationary pair g=k built by DVE
                        tensor.wait_ge(dve, k + 1)
                    if k < NU - 1:
                        tensor.wait_ge(din[k % NB], 16 * (k // NB + 1))
                    xt = xts[k % NB]
                    g = k % NG
                    for h in range(NH):
                        G = k * NH + h
                        if k == NU - 1:
                            # split load: chunk 0 needs half A, chunk 1 both
                            if h == 0:
                                tensor.wait_ge(din[k % NB], 16 * (k // NB + 1))
                            else:
                                tensor.wait_ge(dl, 16)
                        if G >= NPS:
                            # psum buffer free once silu of chunk G-NPS done
                            tensor.wait_ge(act, G - NPS + 1)
                        ps = pss[G % NPS]
                        for c2 in range(PC // NC_CHUNK):
                            c0 = h * PC + c2 * NC_CHUNK   # within the tile
                            p0 = c2 * NC_CHUNK            # within the psum buf
                            if c0 == 0:
                                # block -1 is the causal zero block: psum col
                                # 0 gets no prev contribution. cur starts the
                                # group (zeroes the whole 512-col bank).
                                mm = tensor.matmul(
                                    ps[:, 0:NC_CHUNK],
                                    wmat(g, 1),
                                    xt[:, 0:NC_CHUNK],
                                    start=True,
                                    stop=False,
                                    skip_group_check=True,
                                )
                                mm = tensor.matmul(
                                    ps[:, 1:NC_CHUNK],
                                    wmat(g, 0),
                                    xt[:, 0 : NC_CHUNK - 1],
                                    start=False,
                                    stop=True,
                                    skip_group_check=True,
                                )
                            else:
                                mm = tensor.matmul(
                                    ps[:, p0 : p0 + NC_CHUNK],
                                    wmat(g, 0),
                                    xt[:, c0 - 1 : c0 - 1 + NC_CHUNK],
                                    start=True,
                                    stop=False,
                                    skip_group_check=True,
                                )
                                mm = tensor.matmul(
                                    ps[:, p0 : p0 + NC_CHUNK],
                                    wmat(g, 1),
                                    xt[:, c0 : c0 + NC_CHUNK],
                                    start=False,
                                    stop=True,
                                    skip_group_check=True,
                                )
                        mm.then_inc(pe)

            @block.scalar
            def _(scalar):
                func = getattr(AF, _ACT_FUNC)

                def store_chunk(G):
                    # runs while the next activation occupies the engine, so
                    # chunk G's completion inc has already propagated: the
                    # wait is ~free and the ~900ns semaphore latency stays
                    # off the activation chain. Chunk-granular stores keep
                    # the output queue only ~1 chunk behind the data.
                    k, h = G // NH, G % NH
                    scalar.wait_ge(act, G + 1)
                    scalar.dma_start(
                        out=y_d[k * 128 : (k + 1) * 128, h * PC : (h + 1) * PC],
                        in_=yts[k % NB][:, h * PC : (h + 1) * PC],
                    ).then_inc(dout[k % NB], 16)

                for k in range(NU):
                    for h in range(NH):
                        G = k * NH + h
                        scalar.wait_ge(pe, G + 1)
                        if h == 0 and k >= NB:
                            # yt slot's previous stores (tile k-NB) done;
                            # total-count gate, so the two chunks' incs
                            # mixing on one semaphore is unambiguous
                            scalar.wait_ge(dout[k % NB], 16 * NH * (k // NB))
                        scalar.activation(
                            out=yts[k % NB][:, h * PC : (h + 1) * PC],
                            in_=pss[G % NPS][:, :],
                            func=func,
                            bias=0.0 if func == AF.Copy else scsb[:, SCC - 1 : SCC],
                            scale=1.0,
                        ).then_inc(act)
                        if G >= 1:
                            store_chunk(G - 1)
                store_chunk(NU * NH - 1)
                for i in range(NB):
                    n_tiles = len([k for k in range(NU) if k % NB == i])
                    scalar.wait_ge(dout[i], 16 * NH * n_tiles)

    return nc


def _scalar_table(ws):
    """ws: (HC, K) f32 -> [128, NG*(2K-1)+1] f32 per-diagonal scalar columns.

    Partition index q = 4*rho + p_in; the device scatters column s onto the
    shifted diagonal [q, q+delta]. Cur-block (delta = p_out - p_in in 0..K-1)
    carries tap i = K-1-delta, valid while (q%P)+delta <= P-1; prev-block
    (delta = -1..-(K-1)) carries tap i = -delta-1, valid while (q%P) >= -delta.
    The final column is zeros (Silu bias operand).
    """
    q = np.arange(128)
    sc = np.zeros((128, NG * (2 * K - 1) + 1), np.float32)
    for g in range(NG):
        ch = ws[RPU * g : RPU * (g + 1)]          # (RPU, K)
        wq = ch[q // P, :]                        # (128, K) per-partition taps
        for delta in range(K):
            sc[:, g * K + delta] = wq[:, K - 1 - delta] * ((q % P) + delta <= P - 1)
        for dp in range(1, K):
            sc[:, NG * K + g * (K - 1) + dp - 1] = wq[:, dp - 1] * ((q % P) >= dp)
    return sc


def kernel(x, weight):
    global _last_results
    from concourse.bass_utils import run_bass_kernel_spmd

    x = np.asarray(x, dtype=np.float32)
    weight = np.asarray(weight, dtype=np.float32)

    nc = _build_program()

    in_maps = []
    for core in range(N_CORES):
        sl = slice(core * HC, (core + 1) * HC)
        # [B, S, HC] -> [B, HC, S] -> [ROWS, S], row r = b*HC + c
        xc = x[:, :, sl].transpose(0, 2, 1).reshape(ROWS, S)
        # phase split: row 4r+p, col j = x[r, 4j+p]
        xs = np.ascontiguousarray(
            xc.reshape(ROWS, J, P).transpose(0, 2, 1).reshape(ROWS * P, J)
        ).astype(BF16)
        in_maps.append({"x": xs, "sc": _scalar_table(weight[sl, :])})

    res = run_bass_kernel_spmd(nc, in_maps, list(range(N_CORES)))
    _last_results = res

    out = np.empty((B, S, H), np.float32)
    for core in range(N_CORES):
        sl = slice(core * HC, (core + 1) * HC)
        yc = np.asarray(res.results[core]["y"], dtype=np.float32)
        # undo phase split, then row-major [B, HC, S] -> [B, S, HC]
        yc = yc.reshape(ROWS, P, J).transpose(0, 2, 1).reshape(B, HC, S)
        out[:, :, sl] = yc.transpose(0, 2, 1)
    return out
